# revision 8
# baseline (speedup 1.0000x reference)
"""ChebyKAN linear layer on 8 Trainium2 NeuronCores.

Math: y[b,j] = sum_{i,k} T_k(tanh(x[b,i])) * C[i,j,k],  k = 0..8.

  - Device computes the PRODUCT basis Q = [T1, T1^2, T1*T2, T2^2, T2*T3,
    T3^2, T3*T4, T4^2]. The recurrence chain T1,t2,t3,t4 stays f32
    (ACT affines + DVE products); the PE-facing Q tiles are bf16:
    squares come straight off ACT (bf16 output is full rate), the
    products Q1,Q3,Q5,Q7 are rounded f32->bf16 by gpsimd DMA-casts
    (RNE, zero compute-engine cost). Since T_2m = 2*Q_2m - 1 and
    T_2m+1 = 2*Q_2m+1 - T1, affine corrections fold into host-side
    coefficients:
       A_1 = C_1 - C_3 - C_5 - C_7,  A_k = 2*C_k (k>=2),
       bias_j = sum_i (C_0 - C_2 - C_4 - C_6 - C_8)[i,j]
    (bias added during PSUM eviction).
  - The k>=1 contraction is a (2048 x 4096) @ (4096 x 512) bf16 matmul
    per core (fp32 PSUM accumulate; fwl weight loads; measured steady
    issue period ~216 ns/matmul = the N=512 floor). rel err ~3e-3.
  - Coefficient chunks are ordered k-major (c = (k-1)*4 + ic) so the PE
    consumption order matches the basis production order (tanh first).

Sharding: data-parallel over Bv (16384 -> 8 x 2048), cheby_coeffs
replicated (host-relaid-out, bf16). Host pre-transposes x so the
contraction index i lands on SBUF partitions.
"""

import json as _json

import numpy as np

# ---------------------------------------------------------------------------
# Container workarounds (inlined so kernel.py is self-contained):
#  1. walrus here refuses instructions carrying >1 sem-wait; hoist excess
#     waits onto NoOps inserted before the offender (same engine queue).
#  2. TileContext tail drain accumulates one wait per logical processor;
#     pre-split them the same way.
# ---------------------------------------------------------------------------

import concourse.bass as bass
import concourse.tile as tile
from concourse import mybir
from concourse._compat import with_exitstack
from concourse.bass_utils import run_bass_kernel_spmd
from concourse.vector_clock import ScopedClock, VectorClock

_MAX_WAITS = 1


def _legalize_bir_json(raw: bytes) -> bytes:
    bir = _json.loads(raw)
    changed = False
    for fn in bir.get("functions", []):
        for blk in fn.get("blocks", []):
            out = []
            for inst in blk.get("instructions", []):
                si = inst.get("sync_info")
                waits = (si or {}).get("on_wait") or []
                if len(waits) > _MAX_WAITS:
                    changed = True
                    excess, keep = waits[:-_MAX_WAITS], waits[-_MAX_WAITS:]
                    for j, w in enumerate(excess):
                        out.append(
                            {
                                "debug": inst.get("debug", 0),
                                "engine": inst["engine"],
                                "ins": [],
                                "name": f"{inst['name']}--w{j}",
                                "opcode": "NoOp",
                                "outs": [],
                                "sync_info": {"on_update": [], "on_wait": [w]},
                                "text_hint": "wait_split",
                            }
                        )
                    si["on_wait"] = keep
                out.append(inst)
            blk["instructions"] = out
    return _json.dumps(bir).encode() if changed else raw


def _patched_drain_and_barrier(self, tick_clock, wait_clock):
    gc = tick_clock.global_clock
    n = len(gc)
    for proc in range(n):
        t = gc[proc]
        if t <= 0:
            continue
        vec = [0] * n
        vec[proc] = t
        nop = self.nc.sync.nop(nofuse=True, hint="tail_drain_split")
        wait_clock.add_sem_waits(nop.ins, ScopedClock({None: VectorClock(vec)}))
    self.nc.sync.drain()
    self.nc.all_engine_barrier()
    assert self.sems is not None
    popped = self.nc._tile_sem_poison_stack.pop()
    assert popped is self._sem_poison
    self.nc.clear_and_free_semaphores(list(self.sems.allocated().values()))
    self.nc.all_engine_barrier()


def _apply_patches():
    if getattr(bass.Bass, "_cheby_patched", False):
        return
    orig = bass.Bass.to_json_bytes

    def patched(self, *a, **kw):
        return _legalize_bir_json(orig(self, *a, **kw))

    bass.Bass.to_json_bytes = patched
    tile.TileContext._drain_and_barrier = _patched_drain_and_barrier
    bass.Bass._cheby_patched = True


_apply_patches()

# ---------------------------------------------------------------------------
# Problem constants (hardcoded per the harness contract)
# ---------------------------------------------------------------------------
NCORES = 8
BV, DIM, K = 16384, 512, 9
BC = BV // NCORES          # 2048 rows per core
SC = 512                   # b-superchunk width
NSC = BC // SC             # 4 superchunks per core
NIC = DIM // 128           # 4 i-chunks
NCH = NIC * (K - 1)        # 32 contraction chunks, c = (k-1)*4 + ic

F32 = mybir.dt.float32
BF16 = mybir.dt.bfloat16
AFT = mybir.ActivationFunctionType
ALU = mybir.AluOpType


def _build_nc():
    nc = bass.Bass()
    xt_d = nc.dram_tensor("xt", (128, NSC, NIC * SC), F32, kind="ExternalInput")
    cm_d = nc.dram_tensor("cmat", (128, NCH * DIM), BF16, kind="ExternalInput")
    bi_d = nc.dram_tensor("bias", (128, DIM), F32, kind="ExternalInput")
    y_d = nc.dram_tensor("y", (128, NSC * NSC * DIM), F32, kind="ExternalOutput")

    @with_exitstack
    def kern(ctx, tc):
        nc = tc.nc
        cpool = ctx.enter_context(tc.tile_pool(name="cmat", bufs=1))
        bpool = ctx.enter_context(tc.tile_pool(name="bias", bufs=1))
        xpool = ctx.enter_context(tc.tile_pool(name="x", bufs=2))
        x0pool = ctx.enter_context(tc.tile_pool(name="x0", bufs=1))
        upool = ctx.enter_context(tc.tile_pool(name="u", bufs=4))
        tpool = ctx.enter_context(tc.tile_pool(name="basis", bufs=48))
        ppool = ctx.enter_context(tc.tile_pool(name="ps", bufs=4, space="PSUM"))
        ypool = ctx.enter_context(tc.tile_pool(name="y", bufs=4))

        # superchunk-0 x rides the scalar queue ahead of everything;
        # coefficient chunks (k-major, c = (k-1)*4 + ic) stream on sync
        # in 8 groups of 4, flat-contiguous per partition for cheap
        # descriptors. The first group is exactly the k=1 chunks the
        # first four matmuls need.
        x0 = x0pool.tile([128, SC], F32, tag="x0")
        nc.scalar.dma_start(x0[:], xt_d[:, 0, 0:SC])
        x3 = x0pool.tile([128, (NIC - 1) * SC], F32, tag="x3")
        nc.scalar.dma_start(x3[:], xt_d[:, 0, SC : NIC * SC])

        cm_tiles = []
        G = 4
        for g in range(NCH // G):
            cmt = cpool.tile([128, G * DIM], BF16, tag=f"cmat{g}", name=f"cm{g}")
            nc.sync.dma_start(cmt[:], cm_d[:, g * G * DIM : (g + 1) * G * DIM])
            for jj in range(G):
                cm_tiles.append(cmt[:, jj * DIM : (jj + 1) * DIM])
        bi = bpool.tile([128, DIM], F32, tag="bias")
        nc.sync.dma_start(bi[:], bi_d[:])
        negone = bpool.tile([128, 1], F32, tag="negone")
        nc.gpsimd.memset(negone[:], -1.0)

        def utile(tag, s, ic):
            return upool.tile([128, SC], F32, tag=tag, name=f"{tag}_{s}_{ic}")

        for s in range(NSC):
            # superchunk of x (superchunk 0 was loaded up front, split
            # so the ic0 tanh chain starts as early as possible)
            if s == 0:
                xs = [x0[:]] + [
                    x3[:, (ic - 1) * SC : ic * SC] for ic in range(1, NIC)
                ]
            else:
                xt = xpool.tile([128, NIC * SC], F32, tag="x", name=f"x_{s}")
                nc.scalar.dma_start(xt[:], xt_d[:, s, :])
                xs = [xt[:, ic * SC : (ic + 1) * SC] for ic in range(NIC)]

            # basis production, level-major across i-chunks so the PE's
            # k-major consumption can start right after the tanh level.
            T1 = [utile("T1", s, ic) for ic in range(NIC)]
            t2a = [utile("t2a", s, ic) for ic in range(NIC)]
            t2 = [utile("t2", s, ic) for ic in range(NIC)]
            Q3 = [utile("Q3f", s, ic) for ic in range(NIC)]
            t3a = [utile("t3a", s, ic) for ic in range(NIC)]
            t3 = [utile("t3", s, ic) for ic in range(NIC)]
            t4a = [utile("t4a", s, ic) for ic in range(NIC)]
            t4 = [utile("t4", s, ic) for ic in range(NIC)]
            Q5 = [utile("Q5f", s, ic) for ic in range(NIC)]
            Q7 = [utile("Q7f", s, ic) for ic in range(NIC)]
            B = [
                [
                    tpool.tile([128, SC], BF16, tag="basis", name=f"B{s}_{k}_{ic}")
                    for ic in range(NIC)
                ]
                for k in range(8)
            ]
            for ic in range(NIC):
                nc.scalar.activation(T1[ic][:], xs[ic], AFT.Tanh)
                if s == 0:
                    nc.scalar.activation(B[0][ic][:], xs[ic], AFT.Tanh)
            if s > 0:
                for ic in range(NIC):
                    nc.gpsimd.dma_start(B[0][ic][:], T1[ic][:])  # Q1 cast
            for ic in range(NIC):
                nc.scalar.activation(B[1][ic][:], T1[ic][:], AFT.Square)  # Q2
            for ic in range(NIC):
                nc.vector.tensor_mul(t2a[ic][:], T1[ic][:], T1[ic][:])
            for ic in range(NIC):
                nc.scalar.activation(
                    t2[ic][:], t2a[ic][:], AFT.Identity, scale=2.0, bias=negone[:]
                )
            for ic in range(NIC):
                nc.vector.tensor_mul(Q3[ic][:], T1[ic][:], t2[ic][:])
            for ic in range(NIC):
                nc.gpsimd.dma_start(B[2][ic][:], Q3[ic][:])  # Q3 cast
            for ic in range(NIC):
                nc.vector.tensor_add(t3a[ic][:], Q3[ic][:], Q3[ic][:])
            for ic in range(NIC):
                nc.vector.tensor_sub(t3[ic][:], t3a[ic][:], T1[ic][:])
            for ic in range(NIC):
                nc.scalar.activation(B[3][ic][:], t2[ic][:], AFT.Square)  # Q4
            for ic in range(NIC):
                nc.vector.tensor_mul(t4a[ic][:], t2[ic][:], t2[ic][:])
            for ic in range(NIC):
                nc.scalar.activation(
                    t4[ic][:], t4a[ic][:], AFT.Identity, scale=2.0, bias=negone[:]
                )
            for ic in range(NIC):
                nc.vector.tensor_mul(Q5[ic][:], t2[ic][:], t3[ic][:])
            for ic in range(NIC):
                nc.gpsimd.dma_start(B[4][ic][:], Q5[ic][:])  # Q5 cast
            for ic in range(NIC):
                nc.scalar.activation(B[5][ic][:], t3[ic][:], AFT.Square)  # Q6
            for ic in range(NIC):
                nc.vector.tensor_mul(Q7[ic][:], t3[ic][:], t4[ic][:])
            for ic in range(NIC):
                nc.gpsimd.dma_start(B[6][ic][:], Q7[ic][:])  # Q7 cast
            for ic in range(NIC):
                nc.scalar.activation(B[7][ic][:], t4[ic][:], AFT.Square)  # Q8

            for bc in range(SC // 128):
                ps = ppool.tile([128, DIM], F32, tag="ps")
                for c in range(NCH):
                    km1, ic = divmod(c, NIC)
                    lhsT = B[km1][ic][:, bc * 128 : (bc + 1) * 128]
                    nc.tensor.matmul(
                        ps[:],
                        lhsT,
                        cm_tiles[c],
                        start=(c == 0),
                        stop=(c == NCH - 1),
                    )
                yt = ypool.tile([128, DIM], F32, tag="y", name=f"y_{s}_{bc}")
                nc.vector.tensor_add(yt[:], ps[:], bi[:])
                g = s * NSC + bc
                nc.sync.dma_start(y_d[:, g * DIM : (g + 1) * DIM], yt[:])

    with tile.TileContext(nc) as tc:
        kern(tc)
    return nc


_NC_CACHE = None


def _get_nc():
    global _NC_CACHE
    if _NC_CACHE is None:
        _NC_CACHE = _build_nc()
    return _NC_CACHE


def _prep_inputs(x, cheby_coeffs):
    import ml_dtypes

    C = np.asarray(cheby_coeffs, dtype=np.float32)
    # product-basis coefficient transform (see module docstring)
    A = np.empty((DIM, DIM, K - 1), np.float32)
    A[:, :, 0] = C[:, :, 1] - C[:, :, 3] - C[:, :, 5] - C[:, :, 7]
    for k in range(2, K):
        A[:, :, k - 1] = 2.0 * C[:, :, k]
    bias_j = (
        (C[:, :, 0] - C[:, :, 2] - C[:, :, 4] - C[:, :, 6] - C[:, :, 8])
        .sum(axis=0, dtype=np.float64)
        .astype(np.float32)
    )
    # contraction chunk c = (k-1)*4 + ic holds A[ic*128:(ic+1)*128, :, k];
    # flat layout (128, NCH*DIM), contiguous per partition line
    cmat = np.empty((NCH, 128, DIM), np.float32)
    for k in range(1, K):
        for ic in range(NIC):
            cmat[(k - 1) * NIC + ic] = A[ic * 128 : (ic + 1) * 128, :, k - 1]
    cm2 = np.ascontiguousarray(cmat.transpose(1, 0, 2).reshape(128, NCH * DIM)).astype(
        ml_dtypes.bfloat16
    )
    bias = np.ascontiguousarray(np.broadcast_to(bias_j, (128, DIM)))
    # x layout (128, NSC, NIC*SC): [p, s, ic*SC+b]
    xc = np.asarray(x, dtype=np.float32).reshape(NCORES, BC, NIC, 128)
    in_maps = []
    for c in range(NCORES):
        # (BC, NIC, 128) -> [p, s, ic, b]
        xv = xc[c].reshape(NSC, SC, NIC, 128).transpose(3, 0, 2, 1)
        in_maps.append(
            {
                "xt": np.ascontiguousarray(xv).reshape(128, NSC, NIC * SC),
                "cmat": cm2,
                "bias": bias,
            }
        )
    return in_maps


def kernel(x, cheby_coeffs, _trace=False, _tmpdir=None):
    nc = _get_nc()
    in_maps = _prep_inputs(x, cheby_coeffs)
    res = run_bass_kernel_spmd(
        nc,
        in_maps,
        core_ids=list(range(NCORES)),
        trace=_trace,
        tmpdir=_tmpdir,
    )
    y = np.concatenate(
        [
            r["y"].reshape(128, NSC * NSC, DIM).transpose(1, 0, 2).reshape(BC, DIM)
            for r in res.results
        ],
        axis=0,
    )
    if _trace:
        kernel.last_result = res
    return y


# revision 9
# speedup vs baseline: 1.0347x; 1.0347x over previous
"""ChebyKAN linear layer on 8 Trainium2 NeuronCores.

Math: y[b,j] = sum_{i,k} T_k(tanh(x[b,i])) * C[i,j,k],  k = 0..8.

  - Device computes the PRODUCT basis Q = [T1, T1^2, T1*T2, T2^2, T2*T3,
    T3^2, T3*T4, T4^2]. The recurrence chain T1,t2,t3,t4 stays f32
    (ACT affines + DVE products); the PE-facing Q tiles are bf16:
    squares come straight off ACT (bf16 output is full rate), the
    products Q1,Q3,Q5,Q7 are rounded f32->bf16 by gpsimd DMA-casts
    (RNE, zero compute-engine cost). Since T_2m = 2*Q_2m - 1 and
    T_2m+1 = 2*Q_2m+1 - T1, affine corrections fold into host-side
    coefficients:
       A_1 = C_1 - C_3 - C_5 - C_7,  A_k = 2*C_k (k>=2),
       bias_j = sum_i (C_0 - C_2 - C_4 - C_6 - C_8)[i,j]
    (bias added during PSUM eviction).
  - The k>=1 contraction is a (2048 x 4096) @ (4096 x 512) bf16 matmul
    per core (fp32 PSUM accumulate; fwl weight loads; measured steady
    issue period ~216 ns/matmul = the N=512 floor). rel err ~3e-3.
  - Coefficient chunks are ordered k-major (c = (k-1)*4 + ic) so the PE
    consumption order matches the basis production order (tanh first).

Sharding: data-parallel over Bv (16384 -> 8 x 2048), cheby_coeffs
replicated (host-relaid-out, bf16). Host pre-transposes x so the
contraction index i lands on SBUF partitions.
"""

import json as _json

import numpy as np

# ---------------------------------------------------------------------------
# Container workarounds (inlined so kernel.py is self-contained):
#  1. walrus here refuses instructions carrying >1 sem-wait; hoist excess
#     waits onto NoOps inserted before the offender (same engine queue).
#  2. TileContext tail drain accumulates one wait per logical processor;
#     pre-split them the same way.
# ---------------------------------------------------------------------------

import concourse.bass as bass
import concourse.tile as tile
from concourse import mybir
from concourse._compat import with_exitstack
from concourse.bass_utils import run_bass_kernel_spmd
from concourse.vector_clock import ScopedClock, VectorClock

_MAX_WAITS = 1


def _legalize_bir_json(raw: bytes) -> bytes:
    bir = _json.loads(raw)
    changed = False
    for fn in bir.get("functions", []):
        for blk in fn.get("blocks", []):
            out = []
            for inst in blk.get("instructions", []):
                si = inst.get("sync_info")
                waits = (si or {}).get("on_wait") or []
                if len(waits) > _MAX_WAITS:
                    changed = True
                    excess, keep = waits[:-_MAX_WAITS], waits[-_MAX_WAITS:]
                    for j, w in enumerate(excess):
                        out.append(
                            {
                                "debug": inst.get("debug", 0),
                                "engine": inst["engine"],
                                "ins": [],
                                "name": f"{inst['name']}--w{j}",
                                "opcode": "NoOp",
                                "outs": [],
                                "sync_info": {"on_update": [], "on_wait": [w]},
                                "text_hint": "wait_split",
                            }
                        )
                    si["on_wait"] = keep
                out.append(inst)
            blk["instructions"] = out
    return _json.dumps(bir).encode() if changed else raw


def _patched_drain_and_barrier(self, tick_clock, wait_clock):
    gc = tick_clock.global_clock
    n = len(gc)
    for proc in range(n):
        t = gc[proc]
        if t <= 0:
            continue
        vec = [0] * n
        vec[proc] = t
        nop = self.nc.sync.nop(nofuse=True, hint="tail_drain_split")
        wait_clock.add_sem_waits(nop.ins, ScopedClock({None: VectorClock(vec)}))
    self.nc.sync.drain()
    self.nc.all_engine_barrier()
    assert self.sems is not None
    popped = self.nc._tile_sem_poison_stack.pop()
    assert popped is self._sem_poison
    self.nc.clear_and_free_semaphores(list(self.sems.allocated().values()))
    self.nc.all_engine_barrier()


def _apply_patches():
    if getattr(bass.Bass, "_cheby_patched", False):
        return
    orig = bass.Bass.to_json_bytes

    def patched(self, *a, **kw):
        return _legalize_bir_json(orig(self, *a, **kw))

    bass.Bass.to_json_bytes = patched
    tile.TileContext._drain_and_barrier = _patched_drain_and_barrier
    bass.Bass._cheby_patched = True


_apply_patches()

# ---------------------------------------------------------------------------
# Problem constants (hardcoded per the harness contract)
# ---------------------------------------------------------------------------
NCORES = 8
BV, DIM, K = 16384, 512, 9
BC = BV // NCORES          # 2048 rows per core
SC = 512                   # b-superchunk width
NSC = BC // SC             # 4 superchunks per core
NIC = DIM // 128           # 4 i-chunks
NCH = NIC * (K - 1)        # 32 contraction chunks, c = (k-1)*4 + ic

F32 = mybir.dt.float32
BF16 = mybir.dt.bfloat16
AFT = mybir.ActivationFunctionType
ALU = mybir.AluOpType


def _build_nc():
    nc = bass.Bass()
    xt_d = nc.dram_tensor("xt", (128, NSC, NIC * SC), F32, kind="ExternalInput")
    cm_d = nc.dram_tensor("cmat", (128, NCH * DIM), BF16, kind="ExternalInput")
    bi_d = nc.dram_tensor("bias", (128, DIM), F32, kind="ExternalInput")
    y_d = nc.dram_tensor("y", (128, NSC * NSC * DIM), F32, kind="ExternalOutput")

    @with_exitstack
    def kern(ctx, tc):
        nc = tc.nc
        cpool = ctx.enter_context(tc.tile_pool(name="cmat", bufs=1))
        bpool = ctx.enter_context(tc.tile_pool(name="bias", bufs=1))
        xpool = ctx.enter_context(tc.tile_pool(name="x", bufs=2))
        x0pool = ctx.enter_context(tc.tile_pool(name="x0", bufs=1))
        upool = ctx.enter_context(tc.tile_pool(name="u", bufs=4))
        tpool = ctx.enter_context(tc.tile_pool(name="basis", bufs=48))
        ppool = ctx.enter_context(tc.tile_pool(name="ps", bufs=8, space="PSUM"))
        ypool = ctx.enter_context(tc.tile_pool(name="y", bufs=4))

        # superchunk-0 x rides the scalar queue ahead of everything;
        # coefficient chunks (k-major, c = (k-1)*4 + ic) stream on sync
        # in 8 groups of 4, flat-contiguous per partition for cheap
        # descriptors. The first group is exactly the k=1 chunks the
        # first four matmuls need.
        x0 = x0pool.tile([128, SC], F32, tag="x0")
        nc.scalar.dma_start(x0[:], xt_d[:, 0, 0:SC])
        x3 = x0pool.tile([128, (NIC - 1) * SC], F32, tag="x3")
        nc.scalar.dma_start(x3[:], xt_d[:, 0, SC : NIC * SC])

        cm_tiles = []
        G = 4
        for g in range(NCH // G):
            cmt = cpool.tile([128, G * DIM], BF16, tag=f"cmat{g}", name=f"cm{g}")
            nc.sync.dma_start(cmt[:], cm_d[:, g * G * DIM : (g + 1) * G * DIM])
            for jj in range(G):
                cm_tiles.append(cmt[:, jj * DIM : (jj + 1) * DIM])
        bi = bpool.tile([128, DIM], F32, tag="bias")
        nc.sync.dma_start(bi[:], bi_d[:])
        negone = bpool.tile([128, 1], F32, tag="negone")
        nc.gpsimd.memset(negone[:], -1.0)

        def utile(tag, s, ic):
            return upool.tile([128, SC], F32, tag=tag, name=f"{tag}_{s}_{ic}")

        for s in range(NSC):
            # superchunk of x (superchunk 0 was loaded up front, split
            # so the ic0 tanh chain starts as early as possible)
            if s == 0:
                xs = [x0[:]] + [
                    x3[:, (ic - 1) * SC : ic * SC] for ic in range(1, NIC)
                ]
            else:
                xt = xpool.tile([128, NIC * SC], F32, tag="x", name=f"x_{s}")
                nc.scalar.dma_start(xt[:], xt_d[:, s, :])
                xs = [xt[:, ic * SC : (ic + 1) * SC] for ic in range(NIC)]

            # basis production, level-major across i-chunks so the PE's
            # k-major consumption can start right after the tanh level.
            T1 = [utile("T1", s, ic) for ic in range(NIC)]
            t2a = [utile("t2a", s, ic) for ic in range(NIC)]
            t2 = [utile("t2", s, ic) for ic in range(NIC)]
            Q3 = [utile("Q3f", s, ic) for ic in range(NIC)]
            t3a = [utile("t3a", s, ic) for ic in range(NIC)]
            t3 = [utile("t3", s, ic) for ic in range(NIC)]
            t4a = [utile("t4a", s, ic) for ic in range(NIC)]
            t4 = [utile("t4", s, ic) for ic in range(NIC)]
            Q5 = [utile("Q5f", s, ic) for ic in range(NIC)]
            Q7 = [utile("Q7f", s, ic) for ic in range(NIC)]
            B = [
                [
                    tpool.tile([128, SC], BF16, tag="basis", name=f"B{s}_{k}_{ic}")
                    for ic in range(NIC)
                ]
                for k in range(8)
            ]
            for ic in range(NIC):
                nc.scalar.activation(T1[ic][:], xs[ic], AFT.Tanh)
                if s == 0:
                    nc.scalar.activation(B[0][ic][:], xs[ic], AFT.Tanh)
            if s > 0:
                for ic in range(NIC):
                    nc.gpsimd.dma_start(B[0][ic][:], T1[ic][:])  # Q1 cast
            for ic in range(NIC):
                nc.scalar.activation(B[1][ic][:], T1[ic][:], AFT.Square)  # Q2
            for ic in range(NIC):
                nc.vector.tensor_mul(t2a[ic][:], T1[ic][:], T1[ic][:])
            for ic in range(NIC):
                nc.scalar.activation(
                    t2[ic][:], t2a[ic][:], AFT.Identity, scale=2.0, bias=negone[:]
                )
            for ic in range(NIC):
                nc.vector.tensor_mul(Q3[ic][:], T1[ic][:], t2[ic][:])
            for ic in range(NIC):
                nc.gpsimd.dma_start(B[2][ic][:], Q3[ic][:])  # Q3 cast
            for ic in range(NIC):
                nc.vector.tensor_add(t3a[ic][:], Q3[ic][:], Q3[ic][:])
            for ic in range(NIC):
                nc.vector.tensor_sub(t3[ic][:], t3a[ic][:], T1[ic][:])
            for ic in range(NIC):
                nc.scalar.activation(B[3][ic][:], t2[ic][:], AFT.Square)  # Q4
            for ic in range(NIC):
                nc.vector.tensor_mul(t4a[ic][:], t2[ic][:], t2[ic][:])
            for ic in range(NIC):
                nc.scalar.activation(
                    t4[ic][:], t4a[ic][:], AFT.Identity, scale=2.0, bias=negone[:]
                )
            for ic in range(NIC):
                nc.vector.tensor_mul(Q5[ic][:], t2[ic][:], t3[ic][:])
            for ic in range(NIC):
                nc.gpsimd.dma_start(B[4][ic][:], Q5[ic][:])  # Q5 cast
            for ic in range(NIC):
                nc.scalar.activation(B[5][ic][:], t3[ic][:], AFT.Square)  # Q6
            for ic in range(NIC):
                nc.vector.tensor_mul(Q7[ic][:], t3[ic][:], t4[ic][:])
            for ic in range(NIC):
                nc.gpsimd.dma_start(B[6][ic][:], Q7[ic][:])  # Q7 cast
            for ic in range(NIC):
                nc.scalar.activation(B[7][ic][:], t4[ic][:], AFT.Square)  # Q8

            # c-outer / bc-inner: all four psum banks accumulate in
            # lockstep so the PE consumes each basis tile the moment it is
            # produced (and frees it right after its 4 matmuls).
            pss = [
                ppool.tile([128, DIM], F32, tag="ps", name=f"ps_{s}_{bc}")
                for bc in range(SC // 128)
            ]
            for c in range(NCH):
                km1, ic = divmod(c, NIC)
                for bc in range(SC // 128):
                    nc.tensor.matmul(
                        pss[bc][:],
                        B[km1][ic][:, bc * 128 : (bc + 1) * 128],
                        cm_tiles[c],
                        start=(c == 0),
                        stop=(c == NCH - 1),
                    )
            for bc in range(SC // 128):
                yt = ypool.tile([128, DIM], F32, tag="y", name=f"y_{s}_{bc}")
                nc.vector.tensor_add(yt[:], pss[bc][:], bi[:])
                g = s * NSC + bc
                nc.sync.dma_start(y_d[:, g * DIM : (g + 1) * DIM], yt[:])

    with tile.TileContext(nc) as tc:
        kern(tc)
    return nc


_NC_CACHE = None


def _get_nc():
    global _NC_CACHE
    if _NC_CACHE is None:
        _NC_CACHE = _build_nc()
    return _NC_CACHE


def _prep_inputs(x, cheby_coeffs):
    import ml_dtypes

    C = np.asarray(cheby_coeffs, dtype=np.float32)
    # product-basis coefficient transform (see module docstring)
    A = np.empty((DIM, DIM, K - 1), np.float32)
    A[:, :, 0] = C[:, :, 1] - C[:, :, 3] - C[:, :, 5] - C[:, :, 7]
    for k in range(2, K):
        A[:, :, k - 1] = 2.0 * C[:, :, k]
    bias_j = (
        (C[:, :, 0] - C[:, :, 2] - C[:, :, 4] - C[:, :, 6] - C[:, :, 8])
        .sum(axis=0, dtype=np.float64)
        .astype(np.float32)
    )
    # contraction chunk c = (k-1)*4 + ic holds A[ic*128:(ic+1)*128, :, k];
    # flat layout (128, NCH*DIM), contiguous per partition line
    cmat = np.empty((NCH, 128, DIM), np.float32)
    for k in range(1, K):
        for ic in range(NIC):
            cmat[(k - 1) * NIC + ic] = A[ic * 128 : (ic + 1) * 128, :, k - 1]
    cm2 = np.ascontiguousarray(cmat.transpose(1, 0, 2).reshape(128, NCH * DIM)).astype(
        ml_dtypes.bfloat16
    )
    bias = np.ascontiguousarray(np.broadcast_to(bias_j, (128, DIM)))
    # x layout (128, NSC, NIC*SC): [p, s, ic*SC+b]
    xc = np.asarray(x, dtype=np.float32).reshape(NCORES, BC, NIC, 128)
    in_maps = []
    for c in range(NCORES):
        # (BC, NIC, 128) -> [p, s, ic, b]
        xv = xc[c].reshape(NSC, SC, NIC, 128).transpose(3, 0, 2, 1)
        in_maps.append(
            {
                "xt": np.ascontiguousarray(xv).reshape(128, NSC, NIC * SC),
                "cmat": cm2,
                "bias": bias,
            }
        )
    return in_maps


def kernel(x, cheby_coeffs, _trace=False, _tmpdir=None):
    nc = _get_nc()
    in_maps = _prep_inputs(x, cheby_coeffs)
    res = run_bass_kernel_spmd(
        nc,
        in_maps,
        core_ids=list(range(NCORES)),
        trace=_trace,
        tmpdir=_tmpdir,
    )
    y = np.concatenate(
        [
            r["y"].reshape(128, NSC * NSC, DIM).transpose(1, 0, 2).reshape(BC, DIM)
            for r in res.results
        ],
        axis=0,
    )
    if _trace:
        kernel.last_result = res
    return y
